# revision 1
# baseline (speedup 1.0000x reference)
"""Trainium2 Bass kernel for nn_MultiHeadAttention (B=4, S=2048, D=768, H=12).

Sharding: 8 cores = 4 batches x 2 head-groups (6 heads each).
Per core, everything is computed in transposed layout:
  QT = Wq_g @ x_b^T            [384, 2048]   (bf16, bias folded)
  KT = Wk_g @ x_b^T            [384, 2048]   (bf16, bias folded)
  V  = x_b @ Wv_g^T            [2048, 6*65]  (bf16; per-head 65th column = 1.0)
  per head h, q-chunk qc:
    S^T[k,q]  = K_h Q_h^T      (PE, k on partitions -> softmax denom via matmul)
    E = exp(S^T/64)            (ScalarE, no max-subtraction: |S/64| < ~1)
    attT_aug  = [V_h | 1]^T E  [65, 512]; row 64 = softmax denominator
    attn = attT / denom        (recip + PE partition-broadcast + DVE mult)
  outT_partial = Wo_g @ attn   [768, 2048]  (+bo on g==0 cores)
Host sums the two partial outT per batch and transposes back.

Self-contained: hardcodes all shapes; only imports concourse + numpy.
"""

import os
import sys

import numpy as np
import ml_dtypes

if "/opt/trn_rl_repo" not in sys.path:
    sys.path.insert(0, "/opt/trn_rl_repo")

import concourse.bass as bass
import concourse.bacc as bacc
import concourse.mybir as mybir
import concourse.tile as tile
from concourse.bass_utils import run_bass_kernel_spmd

# Problem dims
B, S, DM, NH, DK = 4, 2048, 768, 12, 64
NCORES = 8
HLOC = 6          # heads per core
GD = HLOC * DK    # 384 head dims per core
P = 128
NXT = DM // P     # 6 contraction tiles over d_model
NPT = GD // P     # 3 partition tiles over per-core head dims
NKT = S // P      # 16 k tiles
QC = 512          # q chunk
NQC = S // QC     # 4
VROW = HLOC * 2 * DK  # 768: per head, 64 data cols + 64 ones cols (denom replication)

F32 = mybir.dt.float32
BF16 = mybir.dt.bfloat16
EXP = mybir.ActivationFunctionType.Exp
NPBF16 = ml_dtypes.bfloat16

_NC_CACHE = {}


def build_nc():
    nc = bacc.Bacc()

    xT = nc.declare_dram_parameter("xT", [DM, S], BF16, isOutput=False)
    wqT = nc.declare_dram_parameter("wqT", [DM, GD], BF16, isOutput=False)
    wkT = nc.declare_dram_parameter("wkT", [DM, GD], BF16, isOutput=False)
    wvT = nc.declare_dram_parameter("wvT", [DM, GD], BF16, isOutput=False)
    woT = nc.declare_dram_parameter("woT", [GD, DM], BF16, isOutput=False)
    pb = nc.declare_dram_parameter("pb", [P, 12], F32, isOutput=False)
    rcb = nc.declare_dram_parameter("rcb", [1, 512], BF16, isOutput=False)
    outT = nc.declare_dram_parameter("outT", [DM, S], F32, isOutput=True)

    with tile.TileContext(nc) as tc:
        with (
            nc.allow_low_precision(reason="bf16 matmul pipeline is intended"),
            tc.tile_pool(name="persist", bufs=1) as pp,
            tc.tile_pool(name="psum", bufs=1, space=bass.MemorySpace.PSUM) as psp,
            tc.tile_pool(name="work", bufs=1) as wp,
        ):
            # ---- loads (interleaved so first projection mms start early) ----
            xt, wq_t, wk_t, wv_t = [], [], [], []
            for i in range(NXT):
                for nm, dram, lst, w in (("wq", wqT, wq_t, GD), ("xt", xT, xt, S)):
                    t = pp.tile([P, w], BF16, tag=f"{nm}{i}", name=f"{nm}{i}")
                    nc.sync.dma_start(t[:], dram[i * P : (i + 1) * P, :])
                    lst.append(t)
            for i in range(NXT):
                for nm, dram, lst, w in (("wk", wkT, wk_t, GD), ("wv", wvT, wv_t, GD)):
                    t = pp.tile([P, w], BF16, tag=f"{nm}{i}", name=f"{nm}{i}")
                    nc.sync.dma_start(t[:], dram[i * P : (i + 1) * P, :])
                    lst.append(t)
            pb_t = pp.tile([P, 12], F32, tag="pb", name="pb_t")
            nc.sync.dma_start(pb_t[:], pb[:])
            rcb_t = pp.tile([1, 512], BF16, tag="rcb", name="rcb_t")
            nc.sync.dma_start(rcb_t[:], rcb[:])
            wo_t = []
            for j in range(NPT):
                t = pp.tile([P, DM], BF16, tag=f"wo{j}", name=f"wo{j}")
                nc.sync.dma_start(t[:], woT[j * P : (j + 1) * P, :])
                wo_t.append(t)

            bv_row = rcb_t[0:1, 0:GD]         # [1, 384]
            ones_row = rcb_t[0:1, GD:GD + P]  # [1, 128] of 1.0

            # ---- Q^T, K^T projections (bf16 out, bias folded) ----
            QT, KT = [], []
            for nm, w, bcol, dst in (("QT", wq_t, 0, QT), ("KT", wk_t, 3, KT)):
                for pt in range(NPT):
                    t = pp.tile([P, S], BF16, tag=f"{nm}{pt}", name=f"{nm}{pt}")
                    dst.append(t)
                for pt in range(NPT):
                    for qc in range(NQC):
                        ps = psp.tile([P, 2 * QC], F32, tag="st", bufs=2, name=f"ps_{nm}{pt}_{qc}")
                        for kt in range(NXT):
                            nc.tensor.matmul(
                                ps[:, 0:QC],
                                w[kt][:, pt * P : (pt + 1) * P],
                                xt[kt][:, qc * QC : (qc + 1) * QC],
                                start=(kt == 0),
                                stop=(kt == NXT - 1),
                            )
                        nc.vector.tensor_scalar_add(
                            dst[pt][:, qc * QC : (qc + 1) * QC],
                            ps[:, 0:QC],
                            pb_t[:, bcol + pt : bcol + pt + 1],
                        )

            # ---- V projection: [2048, 390] bf16, ones col per head ----
            V = []
            for st in range(NKT):
                t = pp.tile([P, VROW], BF16, tag=f"V{st}", name=f"V{st}")
                V.append(t)
            for st in range(NKT):
                ps = psp.tile([P, 2 * QC], F32, tag="st", bufs=2, name=f"ps_v{st}")
                for kt in range(NXT):
                    nc.tensor.matmul(
                        ps[:, 0:GD],
                        xt[kt][:, st * P : (st + 1) * P],
                        wv_t[kt][:],
                        start=(kt == 0),
                        stop=False,
                    )
                # + ones(128,1) x bv(1,384)
                nc.tensor.matmul(
                    ps[:, 0:GD], ones_row, bv_row, start=False, stop=True
                )
                vv = V[st].rearrange("p (h c) -> p h c", h=HLOC)
                nc.vector.tensor_copy(
                    vv[:, :, 0:DK],
                    ps[:, 0:GD].rearrange("p (h c) -> p h c", h=HLOC),
                )
                nc.vector.memset(vv[:, :, DK : 2 * DK], 1.0)

            # ---- attention + o-proj, q-chunk outer ----
            attn = []
            for hp in range(NPT):
                t = pp.tile([P, S], BF16, tag=f"attn{hp}", name=f"attn{hp}")
                attn.append(t)

            def oproj(oqc, mts):
                oqsl = slice(oqc * QC, (oqc + 1) * QC)
                for mt in mts:
                    po = psp.tile([P, QC], F32, tag="ab", bufs=4, name=f"po{mt}_{oqc}")
                    for j in range(NPT):
                        nc.tensor.matmul(
                            po[:],
                            wo_t[j][:, mt * P : (mt + 1) * P],
                            attn[j][:, oqsl],
                            start=(j == 0),
                            stop=(j == NPT - 1),
                        )
                    osb = wp.tile([P, QC], F32, tag="os", bufs=4, name=f"os{mt}_{oqc}")
                    nc.vector.tensor_scalar_add(osb[:], po[:], pb_t[:, 6 + mt : 7 + mt])
                    nc.sync.dma_start(outT[mt * P : (mt + 1) * P, oqsl], osb[:])

            for qc in range(NQC):
                qsl = slice(qc * QC, (qc + 1) * QC)
                for hp in range(NPT):
                    psA = psp.tile([P, QC], F32, tag="ab", bufs=4, name=f"att_a{hp}_{qc}")
                    psB = psp.tile([P, QC], F32, tag="ab", bufs=4, name=f"att_b{hp}_{qc}")
                    hA, hB = 2 * hp, 2 * hp + 1
                    for ktp in range(NKT // 2):
                        stA = psp.tile([P, 2 * QC], F32, tag="st", bufs=2, name=f"stA{hp}_{qc}_{ktp}")
                        stB = psp.tile([P, 2 * QC], F32, tag="st", bufs=2, name=f"stB{hp}_{qc}_{ktp}")
                        for j in range(2):
                            kt = 2 * ktp + j
                            ksl = slice(kt * P, (kt + 1) * P)
                            jsl = slice(j * QC, (j + 1) * QC)
                            # S^T = K_h @ Q_h^T, two heads row-packed in the PE
                            nc.tensor.matmul(
                                stA[:, jsl], KT[hp][0:DK, ksl], QT[hp][0:DK, qsl]
                            )
                            nc.tensor.matmul(
                                stB[:, jsl], KT[hp][DK:P, ksl], QT[hp][DK:P, qsl]
                            )
                        eA = wp.tile([P, 2 * QC], BF16, tag="E", bufs=24, name=f"eA{hp}_{qc}_{ktp}")
                        eB = wp.tile([P, 2 * QC], BF16, tag="E", bufs=24, name=f"eB{hp}_{qc}_{ktp}")
                        nc.scalar.activation(eA[:], stA[:], EXP, scale=1.0 / DK)
                        nc.scalar.activation(eB[:], stB[:], EXP, scale=1.0 / DK)
                        for j in range(2):
                            kt = 2 * ktp + j
                            jsl = slice(j * QC, (j + 1) * QC)
                            nc.tensor.matmul(
                                psA[:],
                                V[kt][:, hA * 2 * DK : (hA + 1) * 2 * DK],
                                eA[:, jsl],
                                start=(kt == 0),
                                stop=(kt == NKT - 1),
                                skip_group_check=True,
                            )
                            nc.tensor.matmul(
                                psB[:],
                                V[kt][:, hB * 2 * DK : (hB + 1) * 2 * DK],
                                eB[:, jsl],
                                start=(kt == 0),
                                stop=(kt == NKT - 1),
                                skip_group_check=True,
                            )
                    # normalize: attn rows = att * recip(denom); denom replicated in rows 64-127
                    nA = wp.tile([DK, QC], F32, tag="nm", bufs=4, name=f"nA{hp}_{qc}")
                    nB = wp.tile([DK, QC], F32, tag="nm", bufs=4, name=f"nB{hp}_{qc}")
                    nc.vector.reciprocal(nA[:], psA[DK:P, :])
                    nc.vector.reciprocal(nB[:], psB[DK:P, :])
                    nc.vector.tensor_mul(attn[hp][0:DK, qsl], psA[0:DK, :], nA[:])
                    nc.vector.tensor_mul(attn[hp][DK:P, qsl], psB[0:DK, :], nB[:])
                    if qc > 0:
                        oproj(qc - 1, [2 * hp, 2 * hp + 1])


            oproj(NQC - 1, list(range(NXT)))

    nc.compile()
    return nc


def make_in_maps(x, Wq, bq, Wk, bk, Wv, bv, Wo, bo):
    in_maps = []
    for c in range(NCORES):
        b, g = c // 2, c % 2
        sl = slice(g * GD, (g + 1) * GD)
        pbv = np.zeros((P, 12), np.float32)
        for j in range(NPT):
            pbv[:, 0 + j] = bq[sl][j * P : (j + 1) * P]
            pbv[:, 3 + j] = bk[sl][j * P : (j + 1) * P]
        if g == 0:
            for j in range(NXT):
                pbv[:, 6 + j] = bo[j * P : (j + 1) * P]
        rcbv = np.zeros((1, 512), NPBF16)
        rcbv[0, :GD] = bv[sl].astype(NPBF16)
        rcbv[0, GD : GD + P] = NPBF16(1.0)
        in_maps.append(
            {
                "xT": np.ascontiguousarray(x[b].T).astype(NPBF16),
                "wqT": np.ascontiguousarray(Wq[sl, :].T).astype(NPBF16),
                "wkT": np.ascontiguousarray(Wk[sl, :].T).astype(NPBF16),
                "wvT": np.ascontiguousarray(Wv[sl, :].T).astype(NPBF16),
                "woT": np.ascontiguousarray(Wo[:, sl].T).astype(NPBF16),
                "pb": pbv,
                "rcb": rcbv,
            }
        )
    return in_maps


def kernel(x, Wq, bq, Wk, bk, Wv, bv, Wo, bo, _trace=False):
    x = np.asarray(x, np.float32)
    args = [np.asarray(a, np.float32) for a in (Wq, bq, Wk, bk, Wv, bv, Wo, bo)]
    if "nc" not in _NC_CACHE:
        _NC_CACHE["nc"] = build_nc()
    nc = _NC_CACHE["nc"]
    in_maps = make_in_maps(x, *args)
    res = run_bass_kernel_spmd(
        nc, in_maps, core_ids=list(range(NCORES)), trace=_trace
    )
    _NC_CACHE["last_result"] = res
    out = np.empty((B, S, DM), np.float32)
    for b in range(B):
        out[b] = (res.results[2 * b]["outT"] + res.results[2 * b + 1]["outT"]).T
    return out

